# revision 1
# baseline (speedup 1.0000x reference)
"""Trainium2 Bass kernel for the LocalAggregator nn.Module.

Reference computation:
    power[p,g]  = -0.5 * d^T Prec_g d          (d = pts[p] - means3D[g])
    within[p,g] = all(|voxel(pts[p]) - voxel(means3D[g])| <= radii[g])
    logits      = where(within & power<=0, exp(power), 0) @ opacities

Device algorithm (everything O(P*G) runs on the NeuronCores):
  * power is a quadratic polynomial in the point coordinates, so it is a
    matmul of 10 point features [x2,y2,z2,xy,yz,xz,x,y,z,1] against
    per-gaussian coefficient columns.
  * the voxel box test is computed EXACTLY by a matmul of one-hot voxel
    index features (value 224) against per-gaussian box indicator columns
    {0,1}: the contribution is 224 * (#axes within).  Folding -3*224 into
    the constant coefficient makes the PSUM accumulator equal
        power + 224*(#within - 3)
    which is exactly `power` for fully-within pairs and <= -224 otherwise,
    so exp() underflows to exactly 0.0 in fp32 (matches the reference's
    hard mask; valid because Prec is PSD so power <= 0).
  * ScalarE evaluates exp from PSUM, then a second matmul contracts the
    weights against opacities:  logits^T[c,p] += opa^T . weight^T.

Sharding: points are sorted into 4 x-slabs x 2 y-halves (2048 points per
core); each core keeps only the gaussians whose voxel box overlaps its
point bounding box (~300-500 of 2048).  Coordinates are re-centered per
core to keep the fp32 quadratic-form cancellation error small.
One-hot rows are packed into the spare rows of the fp32 feature chunk
first; the remainder spills into fp8 chunks of 128 rows.
"""

import numpy as np
import ml_dtypes

import concourse.bass as bass
import concourse.mybir as mybir
import concourse.tile as tile
import concourse.bass2jax as _bass2jax
import concourse.bass_utils as _bass_utils
from concourse.bass_utils import run_bass_kernel_spmd

import json as _json


def _split_waits(bir_json):
    """Walrus in this toolchain rejects instructions carrying more than one
    sync wait ("Too many sync wait commands").  Split every multi-wait
    instruction into a chain of single-wait NoOps on the same engine (program
    order on the engine's sequencer preserves the wait-before-op semantics)."""
    if isinstance(bir_json, (bytes, bytearray)):
        m = _json.loads(bir_json.decode())
    else:
        m = _json.loads(bir_json)
    cnt = 0
    for f in m["functions"]:
        for bb in f["blocks"]:
            new_insts = []
            for inst in bb["instructions"]:
                si = inst.get("sync_info")
                waits = (si or {}).get("on_wait") or []
                if len(waits) > 1:
                    eng = inst.get("engine")
                    for w in waits[:-1]:
                        cnt += 1
                        nop = {
                            "debug": 16,
                            "ins": [],
                            "name": f"I-nopw-{cnt}",
                            "opcode": "NoOp",
                            "outs": [],
                            "sync_info": {"on_update": [], "on_wait": [w]},
                        }
                        if eng is not None:
                            nop["engine"] = eng
                        new_insts.append(nop)
                    si["on_wait"] = [waits[-1]]
                new_insts.append(inst)
            bb["instructions"] = new_insts
    return _json.dumps(m).encode()


_orig_compile_bir_kernel = _bass_utils.compile_bir_kernel.__wrapped__ if hasattr(
    _bass_utils.compile_bir_kernel, "__wrapped__") else _bass_utils.compile_bir_kernel


def _patched_compile_bir_kernel(bir_json, tmpdir, neff_name="file.neff"):
    return _orig_compile_bir_kernel(_split_waits(bir_json), tmpdir, neff_name)


_bass2jax.compile_bir_kernel = _patched_compile_bir_kernel
_bass_utils.compile_bir_kernel = _patched_compile_bir_kernel

GRID = np.float32(0.5)
SCALE_MULT = np.float32(3.0)
MPEN = 224.0  # penalty unit; exact in float8_e4m3 (max 240) and >> 104 (fp32 exp underflow)
N_CORES = 8
FP8_NP = ml_dtypes.float8_e4m3
NQUAD = 10  # quadratic feature rows in chunk 0
PBLK = 1024  # point block per exp/psum tile (2 PSUM banks)
NMM = 512  # matmul moving free dim (fp32 max)

_nc_cache = {}


def _build_bass(P_loc, G_loc, C, n_fp8):
    f32 = mybir.dt.float32
    fp8 = mybir.dt.float8e4
    GT = G_loc // 128
    PCC = P_loc // PBLK
    HB = PBLK // NMM  # halves per point block

    nc = bass.Bass()
    f0_d = nc.dram_tensor("f0", [128, P_loc], f32, kind="ExternalInput")
    w0_d = nc.dram_tensor("w0", [128, G_loc], f32, kind="ExternalInput")
    if n_fp8:
        f1_d = nc.dram_tensor("f1", [128, n_fp8, P_loc], fp8, kind="ExternalInput")
        w1_d = nc.dram_tensor("w1", [128, n_fp8, G_loc], fp8, kind="ExternalInput")
    opa_d = nc.dram_tensor("opa", [128, GT, C], mybir.dt.bfloat16, kind="ExternalInput")
    out_d = nc.dram_tensor("out", [C, P_loc], f32, kind="ExternalOutput")

    with tile.TileContext(nc) as tc:
        with (
            tc.tile_pool(name="singles", bufs=1) as singles,
            tc.tile_pool(name="wpool", bufs=3) as wpool,
            tc.tile_pool(name="opool", bufs=2) as opool,
            tc.tile_pool(name="pp", bufs=2, space="PSUM") as pp,
            tc.tile_pool(name="pl", bufs=2, space="PSUM") as pl,
        ):
            w0_sb = singles.tile([128, G_loc], f32)
            nc.sync.dma_start(out=w0_sb[:], in_=w0_d[:])
            if n_fp8:
                w1_sb = singles.tile([128, n_fp8, G_loc], fp8)
                nc.sync.dma_start(out=w1_sb[:], in_=w1_d[:])
                f1_sb = singles.tile([128, n_fp8, P_loc], fp8)
                nc.sync.dma_start(out=f1_sb[:], in_=f1_d[:])
            opa_sb = singles.tile([128, GT, C], mybir.dt.bfloat16)
            nc.sync.dma_start(out=opa_sb[:], in_=opa_d[:])
            f0_sb = singles.tile([128, P_loc], f32)
            for pcc in range(PCC):
                sl = slice(pcc * PBLK, (pcc + 1) * PBLK)
                nc.sync.dma_start(out=f0_sb[:, sl], in_=f0_d[:, sl])

            for pcc in range(PCC):
                psl = [pl.tile([C, NMM], f32, name=f"psl{h}") for h in range(HB)]
                for gt in range(GT):
                    gsl = slice(gt * 128, (gt + 1) * 128)
                    psp = pp.tile([128, PBLK], f32, name="psp")
                    nch = 1 + n_fp8
                    for h in range(HB):
                        fsl = slice(pcc * PBLK + h * NMM, pcc * PBLK + (h + 1) * NMM)
                        osl = slice(h * NMM, (h + 1) * NMM)
                        nc.tensor.matmul(
                            psp[:, osl], w0_sb[:, gsl], f0_sb[:, fsl],
                            start=True, stop=(nch == 1),
                        )
                    for j in range(n_fp8):
                        for h in range(HB):
                            fsl = slice(pcc * PBLK + h * NMM, pcc * PBLK + (h + 1) * NMM)
                            osl = slice(h * NMM, (h + 1) * NMM)
                            nc.tensor.matmul(
                                psp[:, osl], w1_sb[:, j, gsl], f1_sb[:, j, fsl],
                                start=False, stop=(j == n_fp8 - 1),
                            )
                    wt = wpool.tile([128, PBLK], mybir.dt.bfloat16, name="wt")
                    nc.scalar.activation(
                        out=wt[:], in_=psp[:], func=mybir.ActivationFunctionType.Exp
                    )
                    for h in range(HB):
                        osl = slice(h * NMM, (h + 1) * NMM)
                        nc.tensor.matmul(
                            psl[h][:], opa_sb[:, gt, :], wt[:, osl],
                            start=(gt == 0), stop=(gt == GT - 1),
                        )
                for h in range(HB):
                    osb = opool.tile([C, NMM], f32, name="osb")
                    nc.vector.tensor_copy(out=osb[:], in_=psl[h][:])
                    osl = slice(pcc * PBLK + h * NMM, pcc * PBLK + (h + 1) * NMM)
                    nc.sync.dma_start(out=out_d[:, osl], in_=osb[:])
    return nc


def _prepare(inputs):
    """Host-side O(P+G) prep: sharding, feature/coefficient matrices."""
    pts = np.ascontiguousarray(np.asarray(inputs["pts"], dtype=np.float32))
    means3D = np.ascontiguousarray(np.asarray(inputs["means3D"], dtype=np.float32))
    opac = np.asarray(inputs["opacities"], dtype=np.float32)
    scales = np.asarray(inputs["scales"], dtype=np.float32)
    cov3D = np.asarray(inputs["cov3D"], dtype=np.float32)
    pc_min = np.asarray(inputs["pc_min"], dtype=np.float32)

    P = pts.shape[0]
    G = means3D.shape[0]
    C = opac.shape[1]
    assert P % N_CORES == 0
    P_loc = P // N_CORES

    # integer voxel quantities, identical fp32 arithmetic to the reference
    pts_int = np.floor((pts - pc_min[None, :]) / GRID).astype(np.int32)
    means_int = np.floor((means3D - pc_min[None, :]) / GRID).astype(np.int32)
    radii = np.ceil(scales.max(-1) * SCALE_MULT / GRID).astype(np.int32)
    cov6 = cov3D.reshape(G, 9)[:, [0, 4, 8, 1, 5, 2]].astype(np.float64)

    # spatial sharding: 4 x-slabs (by sorted order) x 2 y-halves
    order = np.argsort(pts_int[:, 0], kind="stable")
    parts = []
    q = P // 4
    for xs in range(4):
        chunk = order[xs * q:(xs + 1) * q]
        sub = chunk[np.argsort(pts_int[chunk, 1], kind="stable")]
        parts.append(sub[: q // 2])
        parts.append(sub[q // 2:])
    perm = np.concatenate(parts)

    cores = []
    gmax = 1
    spill_max = 0
    for ci in range(N_CORES):
        idx = perm[ci * P_loc:(ci + 1) * P_loc]
        pi = pts_int[idx]
        lo = pi.min(axis=0)
        hi = pi.max(axis=0)
        span = hi - lo + 1  # [Sz... order: axis 0=x,1=y,2=z]
        gsel = np.where(
            (means_int[:, 0] >= lo[0] - radii) & (means_int[:, 0] <= hi[0] + radii)
            & (means_int[:, 1] >= lo[1] - radii) & (means_int[:, 1] <= hi[1] + radii)
            & (means_int[:, 2] >= lo[2] - radii) & (means_int[:, 2] <= hi[2] + radii)
        )[0]
        cores.append((idx, lo, hi, gsel))
        gmax = max(gmax, len(gsel))
        S = int(span.sum())
        spill_max = max(spill_max, S - (128 - NQUAD))
    G_loc = int(np.ceil(gmax / 128) * 128)
    n_fp8 = int(np.ceil(max(0, spill_max) / 128))

    free0 = 128 - NQUAD  # one-hot rows available in the fp32 chunk
    KTOT = 128 + n_fp8 * 128

    def row_of(s):  # flat one-hot index -> feature row
        return np.where(s < free0, NQUAD + s, 128 + (s - free0))

    in_maps = []
    for ci in range(N_CORES):
        idx, lo, hi, gsel = cores[ci]
        npts = len(idx)
        gl = len(gsel)
        span = hi - lo + 1
        # axis order for the flat one-hot space: z, x, y (z smallest)
        axes = [2, 0, 1]
        offs = np.zeros(3, np.int64)
        acc = 0
        for a in axes:
            offs[a] = acc
            acc += int(span[a])

        cen = (lo + hi + 1).astype(np.float64) * (0.5 * float(GRID))  # meters
        p64 = pts[idx].astype(np.float64) - cen
        m64 = means3D[gsel].astype(np.float64) - cen

        FH = np.zeros((KTOT, npts), np.float32)
        x, y, z = p64[:, 0], p64[:, 1], p64[:, 2]
        FH[0] = x * x; FH[1] = y * y; FH[2] = z * z
        FH[3] = x * y; FH[4] = y * z; FH[5] = x * z
        FH[6] = x; FH[7] = y; FH[8] = z; FH[9] = 1.0
        tcol = np.arange(npts)
        for a in axes:
            s = offs[a] + (pts_int[idx, a] - lo[a])
            FH[row_of(s), tcol] = MPEN

        WH = np.zeros((KTOT, G_loc), np.float32)
        a_, b_, c_ = cov6[gsel, 0], cov6[gsel, 1], cov6[gsel, 2]
        pxy, pyz, pxz = cov6[gsel, 3], cov6[gsel, 4], cov6[gsel, 5]
        mx, my, mz = m64[:, 0], m64[:, 1], m64[:, 2]
        Amx = a_ * mx + pxy * my + pxz * mz
        Amy = pxy * mx + b_ * my + pyz * mz
        Amz = pxz * mx + pyz * my + c_ * mz
        mAm = mx * Amx + my * Amy + mz * Amz
        WH[0, :gl] = -0.5 * a_; WH[1, :gl] = -0.5 * b_; WH[2, :gl] = -0.5 * c_
        WH[3, :gl] = -pxy; WH[4, :gl] = -pyz; WH[5, :gl] = -pxz
        WH[6, :gl] = Amx; WH[7, :gl] = Amy; WH[8, :gl] = Amz
        WH[9, :gl] = -0.5 * mAm - 3.0 * MPEN
        WH[9, gl:] = -3.0 * MPEN  # padded gaussians: exp(-672) == 0
        for a in axes:
            Sa = int(span[a])
            blo = means_int[gsel, a] - radii[gsel] - lo[a]
            bhi = means_int[gsel, a] + radii[gsel] - lo[a]
            k = np.arange(Sa)[:, None]
            box = ((k >= blo[None, :]) & (k <= bhi[None, :])).astype(np.float32)
            WH[row_of(offs[a] + np.arange(Sa))[:, None], np.arange(gl)[None, :]] = box

        opa_pad = np.zeros((G_loc, C), np.float32)
        opa_pad[:gl] = opac[gsel]

        m = {
            "f0": np.ascontiguousarray(FH[:128]),
            "w0": np.ascontiguousarray(WH[:128]),
            "opa": np.ascontiguousarray(
                opa_pad.reshape(G_loc // 128, 128, C).transpose(1, 0, 2)
            ).astype(ml_dtypes.bfloat16),
        }
        if n_fp8:
            m["f1"] = np.ascontiguousarray(
                FH[128:].reshape(n_fp8, 128, npts).transpose(1, 0, 2)
            ).astype(FP8_NP)
            m["w1"] = np.ascontiguousarray(
                WH[128:].reshape(n_fp8, 128, G_loc).transpose(1, 0, 2)
            ).astype(FP8_NP)
        in_maps.append(m)

    return in_maps, perm, (P, P_loc, G_loc, C, n_fp8)


def _run(inputs, trace=False, **run_kwargs):
    in_maps, perm, (P, P_loc, G_loc, C, n_fp8) = _prepare(inputs)
    key = (P_loc, G_loc, C, n_fp8)
    if key not in _nc_cache:
        _nc_cache[key] = _build_bass(P_loc, G_loc, C, n_fp8)
    nc = _nc_cache[key]
    try:
        res = run_bass_kernel_spmd(
            nc, in_maps, core_ids=list(range(N_CORES)), trace=trace, **run_kwargs
        )
    except ModuleNotFoundError:
        res = run_bass_kernel_spmd(
            nc, in_maps, core_ids=list(range(N_CORES)), trace=False, **run_kwargs
        )
    out = np.empty((P, C), np.float32)
    for ci in range(N_CORES):
        out[perm[ci * P_loc:(ci + 1) * P_loc]] = res.results[ci]["out"].T
    return out, res


def kernel(**inputs):
    return _run(inputs)[0]



# revision 11
# speedup vs baseline: 3.4742x; 3.4742x over previous
"""Trainium2 Bass kernel for the LocalAggregator nn.Module.

Reference computation:
    power[p,g]  = -0.5 * d^T Prec_g d          (d = pts[p] - means3D[g])
    within[p,g] = all(|voxel(pts[p]) - voxel(means3D[g])| <= radii[g])
    logits      = where(within & power<=0, exp(power), 0) @ opacities

Device algorithm (everything O(P*G) runs on the NeuronCores):
  * power is a quadratic polynomial in the point coordinates -> a matmul of
    per-point feature rows against per-gaussian coefficient columns.  Both
    sides are triple-split into bf16 (w=w1+w2+w3 exactly); the six combos
    w1f1,w1f2,w2f1,w2f2,w1f3,w3f1 reproduce fp32-level accuracy (dropped
    terms are O(2^-27 * |w||f|)) at bf16 matmul speed (1 cycle/column).
    (float32r would be as fast, but its walrus lowering poisons any
    subsequent matmul issued with start_tensor_calc=False.)
  * the voxel box test is EXACT via a one-hot matmul in fp8 DoubleRow mode
    (0.5 cycle/column): one-hot voxel rows (value 224) x {0,1} interval
    indicator columns add 224*(#axes within); the constant row carries
    -3*224 so non-within pairs land below exp's fp32 underflow (-104).
  * ScalarE evaluates exp from PSUM into bf16, then the opacity matmul is
    TRANSPOSED: stationary = weights [128g x 128p], moving = opacities
    [128g x 18] -> 18-cycle matmuls accumulating logits [p,18] slices in a
    single PSUM bank.
Sharding: 8 x-columns of 2048 points (one per core); per core 6 y-shards of
[384,384,384,384,256,256] points, each with <=128 exactly-culled gaussians
(occupancy test, not bbox).  Coordinates re-centered per shard.
"""

import numpy as np
import ml_dtypes

import concourse.bass as bass
import concourse.mybir as mybir
import concourse.tile as tile
import concourse.bass2jax as _bass2jax
import concourse.bass_utils as _bass_utils
from concourse.bass_utils import run_bass_kernel_spmd

import json as _json


def _split_waits(bir_json):
    """Walrus in this toolchain rejects instructions carrying more than one
    sync wait ("Too many sync wait commands").  Split every multi-wait
    instruction into a chain of single-wait NoOps on the same engine (program
    order on the engine's sequencer preserves the wait-before-op semantics)."""
    if isinstance(bir_json, (bytes, bytearray)):
        m = _json.loads(bir_json.decode())
    else:
        m = _json.loads(bir_json)
    cnt = 0
    for f in m["functions"]:
        for bb in f["blocks"]:
            new_insts = []
            for inst in bb["instructions"]:
                si = inst.get("sync_info")
                waits = (si or {}).get("on_wait") or []
                if len(waits) > 1:
                    eng = inst.get("engine")
                    for w in waits[:-1]:
                        cnt += 1
                        nop = {
                            "debug": 16,
                            "ins": [],
                            "name": f"I-nopw-{cnt}",
                            "opcode": "NoOp",
                            "outs": [],
                            "sync_info": {"on_update": [], "on_wait": [w]},
                        }
                        if eng is not None:
                            nop["engine"] = eng
                        new_insts.append(nop)
                    si["on_wait"] = [waits[-1]]
                new_insts.append(inst)
            bb["instructions"] = new_insts
    return _json.dumps(m).encode()


_orig_compile_bir_kernel = _bass_utils.compile_bir_kernel.__wrapped__ if hasattr(
    _bass_utils.compile_bir_kernel, "__wrapped__") else _bass_utils.compile_bir_kernel


def _patched_compile_bir_kernel(bir_json, tmpdir, neff_name="file.neff"):
    return _orig_compile_bir_kernel(_split_waits(bir_json), tmpdir, neff_name)


_bass2jax.compile_bir_kernel = _patched_compile_bir_kernel
_bass_utils.compile_bir_kernel = _patched_compile_bir_kernel

GRID = np.float32(0.5)
SCALE_MULT = np.float32(3.0)
MPEN = 224.0  # penalty unit; exact in float8_e4m3, 3*MPEN >> 104 (exp underflow)
N_CORES = 8
FP8_NP = ml_dtypes.float8_e4m3
C = 18
P_CORE = 2048
PATTERN = (384, 384, 384, 384, 256, 256)  # per-core y-shard point counts

_nc_cache = {}


def _build_bass(R, S2, pblocks, gts, n_c):
    """R: bf16 feature rows; S2: fp8 one-hot rows per k-tile; pblocks: per-shard
    point counts; gts: per-shard gaussian tile counts; n_c: C."""
    f32 = mybir.dt.float32
    fp8 = mybir.dt.float8e4
    bf16 = mybir.dt.bfloat16
    DR = mybir.MatmulPerfMode.DoubleRow
    Exp = mybir.ActivationFunctionType.Exp

    NS = len(pblocks)
    NG = sum(gts)
    P = sum(pblocks)
    NB = P // 128
    CFW = 128 * NG          # bf16 coefficient columns
    CF = CFW + P            # + feature columns
    COW = 128 * NG
    CO = COW + P
    OBB = NG * n_c * 2      # opacity bytes per partition

    # split points for the one-hot DMA: shard 0 via HWDGE, rest via SWDGE
    oh_cut = COW + pblocks[0]

    nc = bass.Bass()
    fq_d = nc.dram_tensor("fq", [R, CF], bf16, kind="ExternalInput")
    oh0_d = nc.dram_tensor("oh0", [S2, 2, oh_cut], fp8, kind="ExternalInput")
    oh1_d = nc.dram_tensor("oh1", [S2, 2, CO - oh_cut], fp8, kind="ExternalInput")
    ob_d = nc.dram_tensor("ob", [128, OBB], mybir.dt.uint8, kind="ExternalInput")
    out_d = nc.dram_tensor("out", [128, NB, n_c], f32, kind="ExternalOutput")

    with tile.TileContext(nc) as tc:
        with (
            tc.tile_pool(name="singles", bufs=1) as singles,
            tc.tile_pool(name="wpool", bufs=3) as wpool,
            tc.tile_pool(name="pp", bufs=3, space="PSUM") as pp,
            tc.tile_pool(name="pl", bufs=1, space="PSUM") as pl,
        ):
            fq_sb = singles.tile([R, CF], bf16)
            oh_sb = singles.tile([S2, 2, CO], fp8)
            ob_sb = singles.tile([128, OBB], mybir.dt.uint8)
            osb = singles.tile([128, NB * n_c], f32)

            nc.sync.dma_start(out=fq_sb[:], in_=fq_d[:])
            nc.sync.dma_start(out=oh_sb[:, :, :oh_cut], in_=oh0_d[:])
            nc.sync.dma_start(out=ob_sb[:], in_=ob_d[:])
            nc.gpsimd.dma_start(out=oh_sb[:, :, oh_cut:], in_=oh1_d[:])

            opa = ob_sb[:, 0:OBB].bitcast(bf16)  # [128, NG*C]

            psl = pl.tile([128, NB * n_c], f32, name="psl")

            # per-shard bookkeeping
            goff = [sum(gts[:s]) for s in range(NS)]
            poff = [sum(pblocks[:s]) for s in range(NS)]
            boff = [poff[s] // 128 for s in range(NS)]

            n_final = sum(gts[s] * (pblocks[s] // 128) for s in range(NS))
            fin_i = 0

            psp_tiles = [None] * NS
            wt_tiles = [None] * NS

            pmax = max(pblocks)

            def emit_power(s):
                ps = pblocks[s]
                tiles = []
                for t in range(gts[s]):
                    gi = goff[s] + t
                    psp = pp.tile([128, pmax], f32, name="psp")[:, :ps]
                    nc.tensor.matmul(
                        psp[:],
                        fq_sb[:, 128 * gi:128 * (gi + 1)],
                        fq_sb[:, CFW + poff[s]:CFW + poff[s] + ps],
                        start=True, stop=False,
                    )
                    nc.tensor.matmul(
                        psp[:],
                        oh_sb[:, :, 128 * gi:128 * (gi + 1)],
                        oh_sb[:, :, COW + poff[s]:COW + poff[s] + ps],
                        start=False, stop=True, perf_mode=DR,
                    )
                    tiles.append(psp)
                psp_tiles[s] = tiles

            def emit_exp(s):
                ps = pblocks[s]
                tiles = []
                for t in range(gts[s]):
                    wt = wpool.tile([128, pmax], bf16, name="wt")[:, :ps]
                    nc.scalar.activation(
                        out=wt[:], in_=psp_tiles[s][t][:], func=Exp
                    )
                    tiles.append(wt)
                wt_tiles[s] = tiles

            def emit_final(s):
                nonlocal fin_i
                ps = pblocks[s]
                for t in range(gts[s]):
                    gi = goff[s] + t
                    wt = wt_tiles[s][t]
                    for b in range(ps // 128):
                        cs = (boff[s] + b) * n_c
                        nc.tensor.matmul(
                            psl[:, cs:cs + n_c],
                            wt[:, 128 * b:128 * (b + 1)],
                            opa[:, gi * n_c:(gi + 1) * n_c],
                            start=(fin_i == 0), stop=(fin_i == n_final - 1),
                        )
                        fin_i += 1

            def emit_out(s):
                cs = boff[s] * n_c
                ce = cs + (pblocks[s] // 128) * n_c
                nc.vector.tensor_copy(out=osb[:, cs:ce], in_=psl[:, cs:ce])
                nc.sync.dma_start(
                    out=out_d[:, boff[s]:boff[s] + pblocks[s] // 128, :],
                    in_=osb[:, cs:ce],
                )

            # software pipeline: power mms run ahead; exp as soon as each
            # shard's psum closes; finals+out trail one shard behind.
            emit_power(0)
            emit_exp(0)
            for s in range(1, NS):
                emit_power(s)
                emit_exp(s)
                emit_final(s - 1)
                emit_out(s - 1)
            emit_final(NS - 1)
            emit_out(NS - 1)
    return nc


BF16 = ml_dtypes.bfloat16
# combo i pairs w-part WCOMBO[i] with f-part FCOMBO[i]; the six combos cover
# every product pair down to O(2^-27).
WCOMBO = (0, 0, 1, 1, 0, 2)
FCOMBO = (0, 1, 0, 1, 2, 0)


def _tsplit(x):
    """Exact bf16 triple split of a float64 array: x ~= x1+x2+x3."""
    x = np.asarray(x, np.float64)
    x1 = x.astype(BF16)
    r1 = x - x1.astype(np.float64)
    x2 = r1.astype(BF16)
    x3 = (r1 - x2.astype(np.float64)).astype(BF16)
    return x1, x2, x3


def _prepare(inputs):
    """Host-side O(P+G) prep: sharding, culling, feature/coefficient packing."""
    pts = np.ascontiguousarray(np.asarray(inputs["pts"], dtype=np.float32))
    means3D = np.ascontiguousarray(np.asarray(inputs["means3D"], dtype=np.float32))
    opac = np.asarray(inputs["opacities"], dtype=np.float32)
    scales = np.asarray(inputs["scales"], dtype=np.float32)
    cov3D = np.asarray(inputs["cov3D"], dtype=np.float32)
    pc_min = np.asarray(inputs["pc_min"], dtype=np.float32)

    P, G = pts.shape[0], means3D.shape[0]
    n_c = opac.shape[1]
    assert P == N_CORES * P_CORE

    # integer voxel quantities, identical fp32 arithmetic to the reference
    pts_int = np.floor((pts - pc_min[None, :]) / GRID).astype(np.int32)
    means_int = np.floor((means3D - pc_min[None, :]) / GRID).astype(np.int32)
    radii = np.ceil(scales.max(-1) * SCALE_MULT / GRID).astype(np.int32)
    cov6 = cov3D.reshape(G, 9)[:, [0, 4, 8, 1, 5, 2]].astype(np.float64)
    has_cross = bool(np.abs(cov6[:, 3:]).max() > 0.0)

    a_, b_, c_ = cov6[:, 0], cov6[:, 1], cov6[:, 2]
    pxy, pyz, pxz = cov6[:, 3], cov6[:, 4], cov6[:, 5]

    # spatial sharding: 8 x-columns (by sorted order) -> cores; 6 y-shards each
    order = np.argsort(pts_int[:, 0], kind="stable")
    cores = []
    for xs in range(4):
        chunk = order[xs * 4096:(xs + 1) * 4096]
        sub = chunk[np.argsort(pts_int[chunk, 1], kind="stable")]
        cores.append(sub[:P_CORE])
        cores.append(sub[P_CORE:])

    NS = len(PATTERN)
    poff = [sum(PATTERN[:s]) for s in range(NS)]

    # exact culling + per-shard metadata
    shard_info = []  # [core][shard] -> (idx, gsel, lo, hi)
    gts = [1] * NS
    smax = 1
    for ci in range(N_CORES):
        rows = []
        for s in range(NS):
            idx = cores[ci][poff[s]:poff[s] + PATTERN[s]]
            pi = pts_int[idx]
            lo, hi = pi.min(0), pi.max(0)
            cand = np.where(
                (means_int >= lo - radii[:, None]).all(1)
                & (means_int <= hi + radii[:, None]).all(1)
            )[0]
            keep = [g for g in cand
                    if (np.abs(pi - means_int[g]) <= radii[g]).all(1).any()]
            gsel = np.asarray(keep, dtype=np.int64)
            rows.append((idx, gsel, lo, hi))
            gts[s] = max(gts[s], (max(len(gsel), 1) + 127) // 128)
            smax = max(smax, int((hi - lo + 1).sum()))
        shard_info.append(rows)

    gts = tuple(gts)
    S2 = (smax + 1) // 2
    NG = sum(gts)
    goff = [sum(gts[:s]) for s in range(NS)]
    CFW = 128 * NG
    CF = CFW + P_CORE
    COW = 128 * NG
    CO = COW + P_CORE
    OBB = NG * n_c * 2
    oh_cut = COW + PATTERN[0]

    base_rows = 10 if has_cross else 7  # quad + linear + const
    R = 6 * base_rows

    in_maps = []
    for ci in range(N_CORES):
        FQ = np.zeros((R, CF), BF16)
        OH = np.zeros((S2, 2, CO), FP8_NP)
        OPA = np.zeros((128, NG, n_c), ml_dtypes.bfloat16)

        for s in range(NS):
            idx, gsel, lo, hi = shard_info[ci][s]
            ps = PATTERN[s]
            gl = len(gsel)
            cen = (lo + hi + 1).astype(np.float64) * (0.5 * float(GRID))
            p64 = pts[idx].astype(np.float64) - cen
            m64 = means3D[gsel].astype(np.float64) - cen
            x, y, z = p64[:, 0], p64[:, 1], p64[:, 2]
            mx, my, mz = m64[:, 0], m64[:, 1], m64[:, 2]
            ag, bg, cg = a_[gsel], b_[gsel], c_[gsel]

            if has_cross:
                pxyg, pyzg, pxzg = pxy[gsel], pyz[gsel], pxz[gsel]
                feats = [x * x, y * y, z * z, x * y, y * z, x * z,
                         x, y, z, np.ones_like(x)]
                Amx = ag * mx + pxyg * my + pxzg * mz
                Amy = pxyg * mx + bg * my + pyzg * mz
                Amz = pxzg * mx + pyzg * my + cg * mz
                mAm = mx * Amx + my * Amy + mz * Amz
                coefs = [-0.5 * ag, -0.5 * bg, -0.5 * cg, -pxyg, -pyzg, -pxzg,
                         Amx, Amy, Amz, -0.5 * mAm - 3.0 * MPEN]
            else:
                feats = [x * x, y * y, z * z, x, y, z, np.ones_like(x)]
                mAm = ag * mx * mx + bg * my * my + cg * mz * mz
                coefs = [-0.5 * ag, -0.5 * bg, -0.5 * cg,
                         ag * mx, bg * my, cg * mz, -0.5 * mAm - 3.0 * MPEN]

            fcol = CFW + poff[s]
            # padded gaussian columns: all-zero coefs except const -> exp(-672)=0
            gcol = 128 * goff[s]
            gpad = 128 * gts[s]
            for r in range(base_rows):
                fp = _tsplit(feats[r])
                wp = _tsplit(coefs[r])
                for i in range(6):
                    FQ[i * base_rows + r, fcol:fcol + ps] = fp[FCOMBO[i]]
                    FQ[i * base_rows + r, gcol:gcol + gl] = wp[WCOMBO[i]]
            cr = base_rows - 1  # const row: fill padded gaussian columns
            for i in range(6):
                if WCOMBO[i] == 0:
                    FQ[i * base_rows + cr, gcol + gl:gcol + gpad] = BF16(-3.0 * MPEN)
            # one-hot axes: order z, x, y
            span = (hi - lo + 1).astype(np.int64)
            axes = [2, 0, 1]
            offs = np.zeros(3, np.int64)
            acc = 0
            for ax in axes:
                offs[ax] = acc
                acc += int(span[ax])
            tcol = np.arange(ps)
            for ax in axes:
                flat = offs[ax] + (pts_int[idx, ax] - lo[ax])
                OH[flat % S2, flat // S2, fcol + tcol] = FP8_NP(MPEN)
            for ax in axes:
                sa = int(span[ax])
                blo = np.maximum(means_int[gsel, ax] - radii[gsel] - lo[ax], 0)
                bhi = np.minimum(means_int[gsel, ax] + radii[gsel] - lo[ax], sa - 1)
                k = np.arange(sa)[:, None]
                box = ((k >= blo[None, :]) & (k <= bhi[None, :]))
                flat = offs[ax] + np.arange(sa)
                OH[flat % S2, flat // S2, gcol:gcol + gl] = np.where(
                    box, FP8_NP(1.0), FP8_NP(0.0))
            OPA[:gl, goff[s], :] = opac[gsel].astype(ml_dtypes.bfloat16)
            if gts[s] > 1:
                # split gsel across tiles (gl>128)
                OPA[:, goff[s]:goff[s] + gts[s], :] = 0
                for t in range(gts[s]):
                    seg = gsel[128 * t:128 * (t + 1)]
                    OPA[:len(seg), goff[s] + t, :] = opac[seg].astype(
                        ml_dtypes.bfloat16)
                # redo coefficient columns per tile
                # (handled above only for t=0; rebuild full block)
                for r in range(R):
                    FQ[r, gcol:gcol + gpad] = 0
                OH[:, :, gcol:gcol + gpad] = FP8_NP(0.0)
                for t in range(gts[s]):
                    seg = np.arange(128 * t, min(128 * (t + 1), gl))
                    gc2 = gcol + 128 * t
                    n2 = len(seg)
                    for r in range(base_rows):
                        wp = _tsplit(coefs[r][seg])
                        for i in range(6):
                            FQ[i * base_rows + r, gc2:gc2 + n2] = wp[WCOMBO[i]]
                    for i in range(6):
                        if WCOMBO[i] == 0:
                            FQ[i * base_rows + cr, gc2 + n2:gc2 + 128] = BF16(
                                -3.0 * MPEN)
                    for ax in axes:
                        sa = int(span[ax])
                        blo = np.maximum(
                            means_int[gsel[seg], ax] - radii[gsel[seg]] - lo[ax], 0)
                        bhi = np.minimum(
                            means_int[gsel[seg], ax] + radii[gsel[seg]] - lo[ax],
                            sa - 1)
                        k = np.arange(sa)[:, None]
                        box = ((k >= blo[None, :]) & (k <= bhi[None, :]))
                        flat = offs[ax] + np.arange(sa)
                        OH[flat % S2, flat // S2, gc2:gc2 + n2] = np.where(
                            box, FP8_NP(1.0), FP8_NP(0.0))

        ob = np.zeros((128, OBB), np.uint8)
        ob[:, :NG * n_c * 2] = OPA.reshape(128, NG * n_c).view(np.uint8)
        in_maps.append({
            "fq": FQ,
            "oh0": np.ascontiguousarray(OH[:, :, :oh_cut]),
            "oh1": np.ascontiguousarray(OH[:, :, oh_cut:]),
            "ob": ob,
        })

    perm = np.concatenate([cores[ci] for ci in range(N_CORES)])
    cfg = (R, S2, PATTERN, gts, n_c)
    return in_maps, perm, cfg


def _run(inputs, trace=False, **run_kwargs):
    in_maps, perm, cfg = _prepare(inputs)
    if cfg not in _nc_cache:
        _nc_cache[cfg] = _build_bass(*cfg)
    nc = _nc_cache[cfg]
    try:
        res = run_bass_kernel_spmd(
            nc, in_maps, core_ids=list(range(N_CORES)), trace=trace, **run_kwargs
        )
    except ModuleNotFoundError:
        res = run_bass_kernel_spmd(
            nc, in_maps, core_ids=list(range(N_CORES)), trace=False, **run_kwargs
        )
    P = P_CORE * N_CORES
    n_c = cfg[4]
    out = np.empty((P, n_c), np.float32)
    for ci in range(N_CORES):
        o = res.results[ci]["out"]  # [128, NB, C]
        out[perm[ci * P_CORE:(ci + 1) * P_CORE]] = (
            o.transpose(1, 0, 2).reshape(P_CORE, n_c))
    return out, res


def kernel(**inputs):
    return _run(inputs)[0]


# revision 13
# speedup vs baseline: 3.6222x; 1.0426x over previous
"""Trainium2 Bass kernel for the LocalAggregator nn.Module.

Reference computation:
    power[p,g]  = -0.5 * d^T Prec_g d          (d = pts[p] - means3D[g])
    within[p,g] = all(|voxel(pts[p]) - voxel(means3D[g])| <= radii[g])
    logits      = where(within & power<=0, exp(power), 0) @ opacities

Device algorithm (everything O(P*G) runs on the NeuronCores):
  * power is a quadratic polynomial in the point coordinates -> a matmul of
    per-point feature rows against per-gaussian coefficient columns.  Both
    sides are triple-split into bf16 (w=w1+w2+w3 exactly); the six combos
    w1f1,w1f2,w2f1,w2f2,w1f3,w3f1 reproduce fp32-level accuracy (dropped
    terms are O(2^-27 * |w||f|)) at bf16 matmul speed (1 cycle/column).
    (float32r would be as fast, but its walrus lowering poisons any
    subsequent matmul issued with start_tensor_calc=False.)
  * the voxel box test is EXACT via a one-hot matmul in fp8 DoubleRow mode
    (0.5 cycle/column): one-hot voxel rows (value 224) x {0,1} interval
    indicator columns add 224*(#axes within); the constant row carries
    -3*224 so non-within pairs land below exp's fp32 underflow (-104).
  * ScalarE evaluates exp from PSUM into bf16, then the opacity matmul is
    TRANSPOSED: stationary = weights [128g x 128p], moving = opacities
    [128g x 18] -> 18-cycle matmuls accumulating logits [p,18] slices in a
    single PSUM bank.
Sharding: 8 x-columns of 2048 points (one per core); per core 6 y-shards of
[384,384,384,384,256,256] points, each with <=128 exactly-culled gaussians
(occupancy test, not bbox).  Coordinates re-centered per shard.
"""

import numpy as np
import ml_dtypes

import concourse.bass as bass
import concourse.mybir as mybir
import concourse.tile as tile
import concourse.bass2jax as _bass2jax
import concourse.bass_utils as _bass_utils
from concourse.bass_utils import run_bass_kernel_spmd

import json as _json


def _split_waits(bir_json):
    """Walrus in this toolchain rejects instructions carrying more than one
    sync wait ("Too many sync wait commands").  Split every multi-wait
    instruction into a chain of single-wait NoOps on the same engine (program
    order on the engine's sequencer preserves the wait-before-op semantics)."""
    if isinstance(bir_json, (bytes, bytearray)):
        m = _json.loads(bir_json.decode())
    else:
        m = _json.loads(bir_json)
    cnt = 0
    for f in m["functions"]:
        for bb in f["blocks"]:
            new_insts = []
            for inst in bb["instructions"]:
                si = inst.get("sync_info")
                waits = (si or {}).get("on_wait") or []
                if len(waits) > 1:
                    eng = inst.get("engine")
                    for w in waits[:-1]:
                        cnt += 1
                        nop = {
                            "debug": 16,
                            "ins": [],
                            "name": f"I-nopw-{cnt}",
                            "opcode": "NoOp",
                            "outs": [],
                            "sync_info": {"on_update": [], "on_wait": [w]},
                        }
                        if eng is not None:
                            nop["engine"] = eng
                        new_insts.append(nop)
                    si["on_wait"] = [waits[-1]]
                new_insts.append(inst)
            bb["instructions"] = new_insts
    return _json.dumps(m).encode()


_orig_compile_bir_kernel = _bass_utils.compile_bir_kernel.__wrapped__ if hasattr(
    _bass_utils.compile_bir_kernel, "__wrapped__") else _bass_utils.compile_bir_kernel


def _patched_compile_bir_kernel(bir_json, tmpdir, neff_name="file.neff"):
    return _orig_compile_bir_kernel(_split_waits(bir_json), tmpdir, neff_name)


_bass2jax.compile_bir_kernel = _patched_compile_bir_kernel
_bass_utils.compile_bir_kernel = _patched_compile_bir_kernel

GRID = np.float32(0.5)
SCALE_MULT = np.float32(3.0)
MPEN = 224.0  # penalty unit; exact in float8_e4m3, 3*MPEN >> 104 (exp underflow)
N_CORES = 8
FP8_NP = ml_dtypes.float8_e4m3
C = 18
P_CORE = 2048
PATTERN = (384, 384, 384, 384, 256, 256)  # per-core y-shard point counts

_nc_cache = {}


def _build_bass(R, S2, pblocks, gts, n_c):
    """R: bf16 feature rows; S2: fp8 one-hot rows per k-tile; pblocks: per-shard
    point counts; gts: per-shard gaussian tile counts; n_c: C."""
    f32 = mybir.dt.float32
    fp8 = mybir.dt.float8e4
    bf16 = mybir.dt.bfloat16
    DR = mybir.MatmulPerfMode.DoubleRow
    Exp = mybir.ActivationFunctionType.Exp

    NS = len(pblocks)
    NG = sum(gts)
    P = sum(pblocks)
    NB = P // 128
    CFW = 128 * NG          # bf16 coefficient columns
    CF = CFW + P            # + feature columns
    COW = 128 * NG
    CO = COW + P
    OBB = NG * n_c * 2      # opacity bytes per partition

    # split points for the one-hot DMA: shard 0 via HWDGE, rest via SWDGE
    oh_cut = COW + pblocks[0]

    nc = bass.Bass()
    fq_d = nc.dram_tensor("fq", [R, CF], bf16, kind="ExternalInput")
    oh0_d = nc.dram_tensor("oh0", [S2, 2, oh_cut], fp8, kind="ExternalInput")
    oh1_d = nc.dram_tensor("oh1", [S2, 2, CO - oh_cut], fp8, kind="ExternalInput")
    ob_d = nc.dram_tensor("ob", [128, OBB], mybir.dt.uint8, kind="ExternalInput")
    out_d = nc.dram_tensor("out", [128, NB, n_c], f32, kind="ExternalOutput")

    with tile.TileContext(nc) as tc:
        with (
            tc.tile_pool(name="singles", bufs=1) as singles,
            tc.tile_pool(name="wpool", bufs=3) as wpool,
            tc.tile_pool(name="pp", bufs=3, space="PSUM") as pp,
            tc.tile_pool(name="pl", bufs=1, space="PSUM") as pl,
        ):
            fq_sb = singles.tile([R, CF], bf16)
            oh_sb = singles.tile([S2, 2, CO], fp8)
            ob_sb = singles.tile([128, OBB], mybir.dt.uint8)
            osb = singles.tile([128, NB * n_c], f32)

            nc.sync.dma_start(out=fq_sb[:], in_=fq_d[:])
            nc.sync.dma_start(out=oh_sb[:, :, :oh_cut], in_=oh0_d[:])
            nc.sync.dma_start(out=ob_sb[:], in_=ob_d[:])
            nc.gpsimd.dma_start(out=oh_sb[:, :, oh_cut:], in_=oh1_d[:])

            opa = ob_sb[:, 0:OBB].bitcast(bf16)  # [128, NG*C]

            psl = pl.tile([128, NB * n_c], f32, name="psl")

            # per-shard bookkeeping
            goff = [sum(gts[:s]) for s in range(NS)]
            poff = [sum(pblocks[:s]) for s in range(NS)]
            boff = [poff[s] // 128 for s in range(NS)]

            n_final = sum(gts[s] * (pblocks[s] // 128) for s in range(NS))
            fin_i = 0

            psp_tiles = [None] * NS
            wt_tiles = [None] * NS

            pmax = max(pblocks)

            def emit_power(s):
                ps = pblocks[s]
                tiles = []
                for t in range(gts[s]):
                    gi = goff[s] + t
                    psp = pp.tile([128, pmax], f32, name="psp")[:, :ps]
                    nc.tensor.matmul(
                        psp[:],
                        fq_sb[:, 128 * gi:128 * (gi + 1)],
                        fq_sb[:, CFW + poff[s]:CFW + poff[s] + ps],
                        start=True, stop=False,
                    )
                    nc.tensor.matmul(
                        psp[:],
                        oh_sb[:, :, 128 * gi:128 * (gi + 1)],
                        oh_sb[:, :, COW + poff[s]:COW + poff[s] + ps],
                        start=False, stop=True, perf_mode=DR,
                    )
                    tiles.append(psp)
                psp_tiles[s] = tiles

            def emit_exp(s):
                ps = pblocks[s]
                tiles = []
                for t in range(gts[s]):
                    wt = wpool.tile([128, pmax], bf16, name="wt")[:, :ps]
                    nc.scalar.activation(
                        out=wt[:], in_=psp_tiles[s][t][:], func=Exp
                    )
                    tiles.append(wt)
                wt_tiles[s] = tiles

            def emit_final(s):
                nonlocal fin_i
                ps = pblocks[s]
                for t in range(gts[s]):
                    gi = goff[s] + t
                    wt = wt_tiles[s][t]
                    for b in range(ps // 128):
                        cs = (boff[s] + b) * n_c
                        nc.tensor.matmul(
                            psl[:, cs:cs + n_c],
                            wt[:, 128 * b:128 * (b + 1)],
                            opa[:, gi * n_c:(gi + 1) * n_c],
                            start=(fin_i == 0), stop=(fin_i == n_final - 1),
                        )
                        fin_i += 1

            def emit_out(blk0, blk1, eng):
                """Copy psl block range to SBUF and DMA to DRAM on `eng`'s
                HWDGE queue (spreads SEQ + HWDGE issue cost across engines)."""
                cs, ce = blk0 * n_c, blk1 * n_c
                nc.vector.tensor_copy(out=osb[:, cs:ce], in_=psl[:, cs:ce])
                eng.dma_start(out=out_d[:, blk0:blk1, :], in_=osb[:, cs:ce])

            # software pipeline: power mms run ahead; exp as soon as each
            # shard's psum closes; finals trail one shard behind.  Output
            # leaves in two DMAs: shards 0..NS-2 (issued while the last
            # shard computes, from the DVE queue) and the last shard alone
            # (short tail transfer, from the idle SP queue).
            emit_power(0)
            emit_exp(0)
            for s in range(1, NS):
                emit_power(s)
                emit_exp(s)
                emit_final(s - 1)
            emit_out(0, boff[NS - 1], nc.scalar)
            emit_final(NS - 1)
            emit_out(boff[NS - 1], NB, nc.sync)
    return nc


BF16 = ml_dtypes.bfloat16
# combo i pairs w-part WCOMBO[i] with f-part FCOMBO[i]; the six combos cover
# every product pair down to O(2^-27).
WCOMBO = (0, 0, 1, 1, 0, 2)
FCOMBO = (0, 1, 0, 1, 2, 0)


def _tsplit(x):
    """Exact bf16 triple split of a float64 array: x ~= x1+x2+x3."""
    x = np.asarray(x, np.float64)
    x1 = x.astype(BF16)
    r1 = x - x1.astype(np.float64)
    x2 = r1.astype(BF16)
    x3 = (r1 - x2.astype(np.float64)).astype(BF16)
    return x1, x2, x3


def _prepare(inputs):
    """Host-side O(P+G) prep: sharding, culling, feature/coefficient packing."""
    pts = np.ascontiguousarray(np.asarray(inputs["pts"], dtype=np.float32))
    means3D = np.ascontiguousarray(np.asarray(inputs["means3D"], dtype=np.float32))
    opac = np.asarray(inputs["opacities"], dtype=np.float32)
    scales = np.asarray(inputs["scales"], dtype=np.float32)
    cov3D = np.asarray(inputs["cov3D"], dtype=np.float32)
    pc_min = np.asarray(inputs["pc_min"], dtype=np.float32)

    P, G = pts.shape[0], means3D.shape[0]
    n_c = opac.shape[1]
    assert P == N_CORES * P_CORE

    # integer voxel quantities, identical fp32 arithmetic to the reference
    pts_int = np.floor((pts - pc_min[None, :]) / GRID).astype(np.int32)
    means_int = np.floor((means3D - pc_min[None, :]) / GRID).astype(np.int32)
    radii = np.ceil(scales.max(-1) * SCALE_MULT / GRID).astype(np.int32)
    cov6 = cov3D.reshape(G, 9)[:, [0, 4, 8, 1, 5, 2]].astype(np.float64)
    has_cross = bool(np.abs(cov6[:, 3:]).max() > 0.0)

    a_, b_, c_ = cov6[:, 0], cov6[:, 1], cov6[:, 2]
    pxy, pyz, pxz = cov6[:, 3], cov6[:, 4], cov6[:, 5]

    # spatial sharding: 8 x-columns (by sorted order) -> cores; 6 y-shards each
    order = np.argsort(pts_int[:, 0], kind="stable")
    cores = []
    for xs in range(4):
        chunk = order[xs * 4096:(xs + 1) * 4096]
        sub = chunk[np.argsort(pts_int[chunk, 1], kind="stable")]
        cores.append(sub[:P_CORE])
        cores.append(sub[P_CORE:])

    NS = len(PATTERN)
    poff = [sum(PATTERN[:s]) for s in range(NS)]

    # exact culling + per-shard metadata
    shard_info = []  # [core][shard] -> (idx, gsel, lo, hi)
    gts = [1] * NS
    smax = 1
    for ci in range(N_CORES):
        rows = []
        for s in range(NS):
            idx = cores[ci][poff[s]:poff[s] + PATTERN[s]]
            pi = pts_int[idx]
            lo, hi = pi.min(0), pi.max(0)
            cand = np.where(
                (means_int >= lo - radii[:, None]).all(1)
                & (means_int <= hi + radii[:, None]).all(1)
            )[0]
            keep = [g for g in cand
                    if (np.abs(pi - means_int[g]) <= radii[g]).all(1).any()]
            gsel = np.asarray(keep, dtype=np.int64)
            rows.append((idx, gsel, lo, hi))
            gts[s] = max(gts[s], (max(len(gsel), 1) + 127) // 128)
            smax = max(smax, int((hi - lo + 1).sum()))
        shard_info.append(rows)

    gts = tuple(gts)
    S2 = (smax + 1) // 2
    NG = sum(gts)
    goff = [sum(gts[:s]) for s in range(NS)]
    CFW = 128 * NG
    CF = CFW + P_CORE
    COW = 128 * NG
    CO = COW + P_CORE
    OBB = NG * n_c * 2
    oh_cut = COW + PATTERN[0]

    base_rows = 10 if has_cross else 7  # quad + linear + const
    R = 6 * base_rows

    in_maps = []
    for ci in range(N_CORES):
        FQ = np.zeros((R, CF), BF16)
        OH = np.zeros((S2, 2, CO), FP8_NP)
        OPA = np.zeros((128, NG, n_c), ml_dtypes.bfloat16)

        for s in range(NS):
            idx, gsel, lo, hi = shard_info[ci][s]
            ps = PATTERN[s]
            gl = len(gsel)
            cen = (lo + hi + 1).astype(np.float64) * (0.5 * float(GRID))
            p64 = pts[idx].astype(np.float64) - cen
            m64 = means3D[gsel].astype(np.float64) - cen
            x, y, z = p64[:, 0], p64[:, 1], p64[:, 2]
            mx, my, mz = m64[:, 0], m64[:, 1], m64[:, 2]
            ag, bg, cg = a_[gsel], b_[gsel], c_[gsel]

            if has_cross:
                pxyg, pyzg, pxzg = pxy[gsel], pyz[gsel], pxz[gsel]
                feats = [x * x, y * y, z * z, x * y, y * z, x * z,
                         x, y, z, np.ones_like(x)]
                Amx = ag * mx + pxyg * my + pxzg * mz
                Amy = pxyg * mx + bg * my + pyzg * mz
                Amz = pxzg * mx + pyzg * my + cg * mz
                mAm = mx * Amx + my * Amy + mz * Amz
                coefs = [-0.5 * ag, -0.5 * bg, -0.5 * cg, -pxyg, -pyzg, -pxzg,
                         Amx, Amy, Amz, -0.5 * mAm - 3.0 * MPEN]
            else:
                feats = [x * x, y * y, z * z, x, y, z, np.ones_like(x)]
                mAm = ag * mx * mx + bg * my * my + cg * mz * mz
                coefs = [-0.5 * ag, -0.5 * bg, -0.5 * cg,
                         ag * mx, bg * my, cg * mz, -0.5 * mAm - 3.0 * MPEN]

            fcol = CFW + poff[s]
            # padded gaussian columns: all-zero coefs except const -> exp(-672)=0
            gcol = 128 * goff[s]
            gpad = 128 * gts[s]
            for r in range(base_rows):
                fp = _tsplit(feats[r])
                wp = _tsplit(coefs[r])
                for i in range(6):
                    FQ[i * base_rows + r, fcol:fcol + ps] = fp[FCOMBO[i]]
                    FQ[i * base_rows + r, gcol:gcol + gl] = wp[WCOMBO[i]]
            cr = base_rows - 1  # const row: fill padded gaussian columns
            for i in range(6):
                if WCOMBO[i] == 0:
                    FQ[i * base_rows + cr, gcol + gl:gcol + gpad] = BF16(-3.0 * MPEN)
            # one-hot axes: order z, x, y
            span = (hi - lo + 1).astype(np.int64)
            axes = [2, 0, 1]
            offs = np.zeros(3, np.int64)
            acc = 0
            for ax in axes:
                offs[ax] = acc
                acc += int(span[ax])
            tcol = np.arange(ps)
            for ax in axes:
                flat = offs[ax] + (pts_int[idx, ax] - lo[ax])
                OH[flat % S2, flat // S2, fcol + tcol] = FP8_NP(MPEN)
            for ax in axes:
                sa = int(span[ax])
                blo = np.maximum(means_int[gsel, ax] - radii[gsel] - lo[ax], 0)
                bhi = np.minimum(means_int[gsel, ax] + radii[gsel] - lo[ax], sa - 1)
                k = np.arange(sa)[:, None]
                box = ((k >= blo[None, :]) & (k <= bhi[None, :]))
                flat = offs[ax] + np.arange(sa)
                OH[flat % S2, flat // S2, gcol:gcol + gl] = np.where(
                    box, FP8_NP(1.0), FP8_NP(0.0))
            OPA[:gl, goff[s], :] = opac[gsel].astype(ml_dtypes.bfloat16)
            if gts[s] > 1:
                # split gsel across tiles (gl>128)
                OPA[:, goff[s]:goff[s] + gts[s], :] = 0
                for t in range(gts[s]):
                    seg = gsel[128 * t:128 * (t + 1)]
                    OPA[:len(seg), goff[s] + t, :] = opac[seg].astype(
                        ml_dtypes.bfloat16)
                # redo coefficient columns per tile
                # (handled above only for t=0; rebuild full block)
                for r in range(R):
                    FQ[r, gcol:gcol + gpad] = 0
                OH[:, :, gcol:gcol + gpad] = FP8_NP(0.0)
                for t in range(gts[s]):
                    seg = np.arange(128 * t, min(128 * (t + 1), gl))
                    gc2 = gcol + 128 * t
                    n2 = len(seg)
                    for r in range(base_rows):
                        wp = _tsplit(coefs[r][seg])
                        for i in range(6):
                            FQ[i * base_rows + r, gc2:gc2 + n2] = wp[WCOMBO[i]]
                    for i in range(6):
                        if WCOMBO[i] == 0:
                            FQ[i * base_rows + cr, gc2 + n2:gc2 + 128] = BF16(
                                -3.0 * MPEN)
                    for ax in axes:
                        sa = int(span[ax])
                        blo = np.maximum(
                            means_int[gsel[seg], ax] - radii[gsel[seg]] - lo[ax], 0)
                        bhi = np.minimum(
                            means_int[gsel[seg], ax] + radii[gsel[seg]] - lo[ax],
                            sa - 1)
                        k = np.arange(sa)[:, None]
                        box = ((k >= blo[None, :]) & (k <= bhi[None, :]))
                        flat = offs[ax] + np.arange(sa)
                        OH[flat % S2, flat // S2, gc2:gc2 + n2] = np.where(
                            box, FP8_NP(1.0), FP8_NP(0.0))

        ob = np.zeros((128, OBB), np.uint8)
        ob[:, :NG * n_c * 2] = OPA.reshape(128, NG * n_c).view(np.uint8)
        in_maps.append({
            "fq": FQ,
            "oh0": np.ascontiguousarray(OH[:, :, :oh_cut]),
            "oh1": np.ascontiguousarray(OH[:, :, oh_cut:]),
            "ob": ob,
        })

    perm = np.concatenate([cores[ci] for ci in range(N_CORES)])
    cfg = (R, S2, PATTERN, gts, n_c)
    return in_maps, perm, cfg


def _run(inputs, trace=False, **run_kwargs):
    in_maps, perm, cfg = _prepare(inputs)
    if cfg not in _nc_cache:
        _nc_cache[cfg] = _build_bass(*cfg)
    nc = _nc_cache[cfg]
    try:
        res = run_bass_kernel_spmd(
            nc, in_maps, core_ids=list(range(N_CORES)), trace=trace, **run_kwargs
        )
    except ModuleNotFoundError:
        res = run_bass_kernel_spmd(
            nc, in_maps, core_ids=list(range(N_CORES)), trace=False, **run_kwargs
        )
    P = P_CORE * N_CORES
    n_c = cfg[4]
    out = np.empty((P, n_c), np.float32)
    for ci in range(N_CORES):
        o = res.results[ci]["out"]  # [128, NB, C]
        out[perm[ci * P_CORE:(ci + 1) * P_CORE]] = (
            o.transpose(1, 0, 2).reshape(P_CORE, n_c))
    return out, res


def kernel(**inputs):
    return _run(inputs)[0]


# revision 18
# speedup vs baseline: 3.8392x; 1.0599x over previous
"""Trainium2 Bass kernel for the LocalAggregator nn.Module.

Reference computation:
    power[p,g]  = -0.5 * d^T Prec_g d          (d = pts[p] - means3D[g])
    within[p,g] = all(|voxel(pts[p]) - voxel(means3D[g])| <= radii[g])
    logits      = where(within & power<=0, exp(power), 0) @ opacities

Device algorithm (everything O(P*G) runs on the NeuronCores):
  * power is a quadratic polynomial in the point coordinates -> a matmul of
    per-point feature rows against per-gaussian coefficient columns.  Both
    sides are triple-split into bf16 (w=w1+w2+w3 exactly); the six combos
    w1f1,w1f2,w2f1,w2f2,w1f3,w3f1 reproduce fp32-level accuracy (dropped
    terms are O(2^-27 * |w||f|)) at bf16 matmul speed (1 cycle/column).
    (float32r would be as fast, but its walrus lowering poisons any
    subsequent matmul issued with start_tensor_calc=False.)
  * the voxel box test is EXACT via a one-hot matmul in fp8 DoubleRow mode
    (0.5 cycle/column): one-hot voxel rows (value 224) x {0,1} interval
    indicator columns add 224*(#axes within); the constant row carries
    -3*224 so non-within pairs land below exp's fp32 underflow (-104).
  * ScalarE evaluates exp from PSUM into bf16, then the opacity matmul is
    TRANSPOSED: stationary = weights [128g x 128p], moving = opacities
    [128g x 18] -> 18-cycle matmuls accumulating logits [p,18] slices in a
    single PSUM bank.
Sharding: 8 x-columns of 2048 points (one per core); per core 6 y-shards of
[384,384,384,384,256,256] points, each with <=128 exactly-culled gaussians
(occupancy test, not bbox).  Coordinates re-centered per shard.
"""

import numpy as np
import ml_dtypes

import concourse.bass as bass
import concourse.mybir as mybir
import concourse.tile as tile
import concourse.bass2jax as _bass2jax
import concourse.bass_utils as _bass_utils
from concourse.bass_utils import run_bass_kernel_spmd

import json as _json


def _split_waits(bir_json):
    """Walrus in this toolchain rejects instructions carrying more than one
    sync wait ("Too many sync wait commands").  Split every multi-wait
    instruction into a chain of single-wait NoOps on the same engine (program
    order on the engine's sequencer preserves the wait-before-op semantics)."""
    if isinstance(bir_json, (bytes, bytearray)):
        m = _json.loads(bir_json.decode())
    else:
        m = _json.loads(bir_json)
    cnt = 0
    for f in m["functions"]:
        for bb in f["blocks"]:
            new_insts = []
            for inst in bb["instructions"]:
                si = inst.get("sync_info")
                waits = (si or {}).get("on_wait") or []
                if len(waits) > 1:
                    eng = inst.get("engine")
                    for w in waits[:-1]:
                        cnt += 1
                        nop = {
                            "debug": 16,
                            "ins": [],
                            "name": f"I-nopw-{cnt}",
                            "opcode": "NoOp",
                            "outs": [],
                            "sync_info": {"on_update": [], "on_wait": [w]},
                        }
                        if eng is not None:
                            nop["engine"] = eng
                        new_insts.append(nop)
                    si["on_wait"] = [waits[-1]]
                new_insts.append(inst)
            bb["instructions"] = new_insts
    return _json.dumps(m).encode()


_orig_compile_bir_kernel = _bass_utils.compile_bir_kernel.__wrapped__ if hasattr(
    _bass_utils.compile_bir_kernel, "__wrapped__") else _bass_utils.compile_bir_kernel


def _patched_compile_bir_kernel(bir_json, tmpdir, neff_name="file.neff"):
    return _orig_compile_bir_kernel(_split_waits(bir_json), tmpdir, neff_name)


_bass2jax.compile_bir_kernel = _patched_compile_bir_kernel
_bass_utils.compile_bir_kernel = _patched_compile_bir_kernel

GRID = np.float32(0.5)
SCALE_MULT = np.float32(3.0)
MPEN = 224.0  # penalty unit; exact in float8_e4m3, 3*MPEN >> 104 (exp underflow)
N_CORES = 8
FP8_NP = ml_dtypes.float8_e4m3
C = 18
P_CORE = 2048
PATTERN = (384, 384, 384, 384, 256, 256)  # per-core y-shard point counts

_nc_cache = {}


def _build_bass(R, S2, pblocks, gts, n_c):
    """R: bf16 feature rows; S2: fp8 one-hot rows per k-tile; pblocks: per-shard
    point counts; gts: per-shard gaussian tile counts; n_c: C."""
    f32 = mybir.dt.float32
    fp8 = mybir.dt.float8e4
    bf16 = mybir.dt.bfloat16
    DR = mybir.MatmulPerfMode.DoubleRow
    Exp = mybir.ActivationFunctionType.Exp

    NS = len(pblocks)
    NG = sum(gts)
    P = sum(pblocks)
    NB = P // 128
    OBB = NG * n_c * 2      # opacity bytes per partition
    SR = max(R, S2)         # bundle partition rows

    # byte-bundle layout (per bundle partition row): coefficient sections
    # first, then per-shard feature sections (bf16 features + fp8 one-hot).
    fqw_off = 0                       # [R, NG*128] bf16
    ohw_off = NG * 256                # [S2, NG, 2, 128] fp8
    sh_off = []
    acc = 2 * NG * 256
    for ps in pblocks:
        sh_off.append(acc)            # fq-s at acc (2*ps bytes), oh-s follows
        acc += 4 * ps
    TOT = acc
    # chunk boundaries: [W + shard0 | shards 1-2 | shards 3..]
    cut1 = sh_off[1]
    cut2 = sh_off[3]

    nc = bass.Bass()
    bun_d = nc.dram_tensor("bun", [SR, TOT], mybir.dt.uint8, kind="ExternalInput")
    ob_d = nc.dram_tensor("ob", [128, OBB], mybir.dt.uint8, kind="ExternalInput")
    out_d = nc.dram_tensor("out", [128, NB, n_c], f32, kind="ExternalOutput")

    with tile.TileContext(nc) as tc:
        with (
            tc.tile_pool(name="singles", bufs=1) as singles,
            tc.tile_pool(name="wpool", bufs=3) as wpool,
            tc.tile_pool(name="pp", bufs=3, space="PSUM") as pp,
            tc.tile_pool(name="pl", bufs=1, space="PSUM") as pl,
        ):
            bun = singles.tile([SR, TOT], mybir.dt.uint8)
            ob_sb = singles.tile([128, OBB], mybir.dt.uint8)
            osb = singles.tile([128, NB * n_c], f32)

            nc.sync.dma_start(out=bun[:, :cut1], in_=bun_d[:, :cut1])
            nc.sync.dma_start(out=bun[:, cut1:cut2], in_=bun_d[:, cut1:cut2])
            nc.sync.dma_start(out=bun[:, cut2:], in_=bun_d[:, cut2:])
            nc.gpsimd.dma_start(out=ob_sb[:], in_=ob_d[:])

            opa = ob_sb[:, 0:OBB].bitcast(bf16)  # [128, NG*C]

            def fqw_v(gi):
                return bun[0:R, fqw_off + 256 * gi:fqw_off + 256 * (gi + 1)
                           ].bitcast(bf16)

            def ohw_v(gi):
                return bun[0:S2, ohw_off + 256 * gi:ohw_off + 256 * (gi + 1)
                           ].bitcast(fp8).rearrange("p (two c) -> p two c", two=2)

            def fqf_v(s, ps):
                return bun[0:R, sh_off[s]:sh_off[s] + 2 * ps].bitcast(bf16)

            def ohf_v(s, ps):
                return bun[0:S2, sh_off[s] + 2 * ps:sh_off[s] + 4 * ps
                           ].bitcast(fp8).rearrange("p (two c) -> p two c", two=2)

            # per-shard bookkeeping
            goff = [sum(gts[:s]) for s in range(NS)]
            poff = [sum(pblocks[:s]) for s in range(NS)]
            boff = [poff[s] // 128 for s in range(NS)]

            # logits accumulate in two PSUM tiles so the big head DMA can
            # leave while the last shard is still computing
            nbA = boff[NS - 2]
            pslA = pl.tile([128, nbA * n_c], f32, name="pslA")
            pslB = pl.tile([128, (NB - nbA) * n_c], f32, name="pslB")
            finA = sum(gts[s] * (pblocks[s] // 128) for s in range(NS - 2))
            finB = sum(gts[s] * (pblocks[s] // 128) for s in range(NS - 2, NS))
            fin_i = 0

            psp_tiles = [None] * NS
            wt_tiles = [None] * NS

            pmax = max(pblocks)

            def emit_power(s):
                ps = pblocks[s]
                tiles = []
                for t in range(gts[s]):
                    gi = goff[s] + t
                    psp = pp.tile([128, pmax], f32, name="psp")[:, :ps]
                    nc.tensor.matmul(
                        psp[:], fqw_v(gi), fqf_v(s, ps),
                        start=True, stop=False,
                    )
                    nc.tensor.matmul(
                        psp[:], ohw_v(gi), ohf_v(s, ps),
                        start=False, stop=True, perf_mode=DR,
                    )
                    tiles.append(psp)
                psp_tiles[s] = tiles

            def emit_exp(s):
                ps = pblocks[s]
                tiles = []
                for t in range(gts[s]):
                    wt = wpool.tile([128, pmax], bf16, name="wt")[:, :ps]
                    nc.scalar.activation(
                        out=wt[:], in_=psp_tiles[s][t][:], func=Exp
                    )
                    tiles.append(wt)
                wt_tiles[s] = tiles

            def emit_final(s):
                nonlocal fin_i
                ps = pblocks[s]
                last = s >= NS - 2
                psl, nfin, base = (
                    (pslB, finB, nbA) if last else (pslA, finA, 0))
                if s == NS - 2:
                    fin_i = 0
                for t in range(gts[s]):
                    gi = goff[s] + t
                    wt = wt_tiles[s][t]
                    for b in range(ps // 128):
                        cs = (boff[s] + b - base) * n_c
                        nc.tensor.matmul(
                            psl[:, cs:cs + n_c],
                            wt[:, 128 * b:128 * (b + 1)],
                            opa[:, gi * n_c:(gi + 1) * n_c],
                            start=(fin_i == 0), stop=(fin_i == nfin - 1),
                        )
                        fin_i += 1

            def emit_out(blk0, blk1, eng):
                """Copy psl block range to SBUF and DMA to DRAM on `eng`'s
                HWDGE queue (spreads SEQ + HWDGE issue cost across engines)."""
                cs, ce = blk0 * n_c, blk1 * n_c
                psl = pslB if blk0 >= nbA else pslA
                base = nbA * n_c if blk0 >= nbA else 0
                nc.vector.tensor_copy(
                    out=osb[:, cs:ce], in_=psl[:, cs - base:ce - base])
                eng.dma_start(out=out_d[:, blk0:blk1, :], in_=osb[:, cs:ce])

            # software pipeline: power mms run ahead; exp as soon as each
            # shard's psum closes; finals trail one shard behind.  Output
            # leaves in two DMAs: shards 0..NS-2 (issued while the last
            # shard computes, from the DVE queue) and the last shard alone
            # (short tail transfer, from the idle SP queue).
            emit_power(0)
            emit_exp(0)
            for s in range(1, NS):
                emit_power(s)
                emit_exp(s)
                emit_final(s - 1)
            emit_out(0, nbA, nc.sync)
            emit_final(NS - 1)
            emit_out(nbA, NB, nc.scalar)
    return nc


BF16 = ml_dtypes.bfloat16
# combo i pairs w-part WCOMBO[i] with f-part FCOMBO[i]; the six combos cover
# every product pair down to O(2^-27).
WCOMBO = (0, 0, 1, 1, 0, 2)
FCOMBO = (0, 1, 0, 1, 2, 0)


def _tsplit(x):
    """Exact bf16 triple split of a float64 array: x ~= x1+x2+x3."""
    x = np.asarray(x, np.float64)
    x1 = x.astype(BF16)
    r1 = x - x1.astype(np.float64)
    x2 = r1.astype(BF16)
    x3 = (r1 - x2.astype(np.float64)).astype(BF16)
    return x1, x2, x3


def _prepare(inputs):
    """Host-side O(P+G) prep: sharding, culling, feature/coefficient packing."""
    pts = np.ascontiguousarray(np.asarray(inputs["pts"], dtype=np.float32))
    means3D = np.ascontiguousarray(np.asarray(inputs["means3D"], dtype=np.float32))
    opac = np.asarray(inputs["opacities"], dtype=np.float32)
    scales = np.asarray(inputs["scales"], dtype=np.float32)
    cov3D = np.asarray(inputs["cov3D"], dtype=np.float32)
    pc_min = np.asarray(inputs["pc_min"], dtype=np.float32)

    P, G = pts.shape[0], means3D.shape[0]
    n_c = opac.shape[1]
    assert P == N_CORES * P_CORE

    # integer voxel quantities, identical fp32 arithmetic to the reference
    pts_int = np.floor((pts - pc_min[None, :]) / GRID).astype(np.int32)
    means_int = np.floor((means3D - pc_min[None, :]) / GRID).astype(np.int32)
    radii = np.ceil(scales.max(-1) * SCALE_MULT / GRID).astype(np.int32)
    cov6 = cov3D.reshape(G, 9)[:, [0, 4, 8, 1, 5, 2]].astype(np.float64)
    has_cross = bool(np.abs(cov6[:, 3:]).max() > 0.0)

    a_, b_, c_ = cov6[:, 0], cov6[:, 1], cov6[:, 2]
    pxy, pyz, pxz = cov6[:, 3], cov6[:, 4], cov6[:, 5]

    # spatial sharding: 8 x-columns (by sorted order) -> cores; 6 y-shards each
    order = np.argsort(pts_int[:, 0], kind="stable")
    cores = []
    for xs in range(4):
        chunk = order[xs * 4096:(xs + 1) * 4096]
        sub = chunk[np.argsort(pts_int[chunk, 1], kind="stable")]
        cores.append(sub[:P_CORE])
        cores.append(sub[P_CORE:])

    NS = len(PATTERN)
    poff = [sum(PATTERN[:s]) for s in range(NS)]

    # exact culling + per-shard metadata
    shard_info = []  # [core][shard] -> (idx, gsel, lo, hi)
    gts = [1] * NS
    smax = 1
    for ci in range(N_CORES):
        rows = []
        for s in range(NS):
            idx = cores[ci][poff[s]:poff[s] + PATTERN[s]]
            pi = pts_int[idx]
            lo, hi = pi.min(0), pi.max(0)
            cand = np.where(
                (means_int >= lo - radii[:, None]).all(1)
                & (means_int <= hi + radii[:, None]).all(1)
            )[0]
            keep = [g for g in cand
                    if (np.abs(pi - means_int[g]) <= radii[g]).all(1).any()]
            gsel = np.asarray(keep, dtype=np.int64)
            rows.append((idx, gsel, lo, hi))
            gts[s] = max(gts[s], (max(len(gsel), 1) + 127) // 128)
            smax = max(smax, int((hi - lo + 1).sum()))
        shard_info.append(rows)

    gts = tuple(gts)
    S2 = (smax + 1) // 2
    NG = sum(gts)
    goff = [sum(gts[:s]) for s in range(NS)]
    CFW = 128 * NG
    CF = CFW + P_CORE
    COW = 128 * NG
    CO = COW + P_CORE
    OBB = NG * n_c * 2

    base_rows = 10 if has_cross else 7  # quad + linear + const
    R = 6 * base_rows

    in_maps = []
    for ci in range(N_CORES):
        FQ = np.zeros((R, CF), BF16)
        OH = np.zeros((S2, 2, CO), FP8_NP)
        OPA = np.zeros((128, NG, n_c), ml_dtypes.bfloat16)

        for s in range(NS):
            idx, gsel, lo, hi = shard_info[ci][s]
            ps = PATTERN[s]
            gl = len(gsel)
            cen = (lo + hi + 1).astype(np.float64) * (0.5 * float(GRID))
            p64 = pts[idx].astype(np.float64) - cen
            m64 = means3D[gsel].astype(np.float64) - cen
            x, y, z = p64[:, 0], p64[:, 1], p64[:, 2]
            mx, my, mz = m64[:, 0], m64[:, 1], m64[:, 2]
            ag, bg, cg = a_[gsel], b_[gsel], c_[gsel]

            if has_cross:
                pxyg, pyzg, pxzg = pxy[gsel], pyz[gsel], pxz[gsel]
                feats = [x * x, y * y, z * z, x * y, y * z, x * z,
                         x, y, z, np.ones_like(x)]
                Amx = ag * mx + pxyg * my + pxzg * mz
                Amy = pxyg * mx + bg * my + pyzg * mz
                Amz = pxzg * mx + pyzg * my + cg * mz
                mAm = mx * Amx + my * Amy + mz * Amz
                coefs = [-0.5 * ag, -0.5 * bg, -0.5 * cg, -pxyg, -pyzg, -pxzg,
                         Amx, Amy, Amz, -0.5 * mAm - 3.0 * MPEN]
            else:
                feats = [x * x, y * y, z * z, x, y, z, np.ones_like(x)]
                mAm = ag * mx * mx + bg * my * my + cg * mz * mz
                coefs = [-0.5 * ag, -0.5 * bg, -0.5 * cg,
                         ag * mx, bg * my, cg * mz, -0.5 * mAm - 3.0 * MPEN]

            fcol = CFW + poff[s]
            # padded gaussian columns: all-zero coefs except const -> exp(-672)=0
            gcol = 128 * goff[s]
            gpad = 128 * gts[s]
            for r in range(base_rows):
                fp = _tsplit(feats[r])
                wp = _tsplit(coefs[r])
                for i in range(6):
                    FQ[i * base_rows + r, fcol:fcol + ps] = fp[FCOMBO[i]]
                    FQ[i * base_rows + r, gcol:gcol + gl] = wp[WCOMBO[i]]
            cr = base_rows - 1  # const row: fill padded gaussian columns
            for i in range(6):
                if WCOMBO[i] == 0:
                    FQ[i * base_rows + cr, gcol + gl:gcol + gpad] = BF16(-3.0 * MPEN)
            # one-hot axes: order z, x, y
            span = (hi - lo + 1).astype(np.int64)
            axes = [2, 0, 1]
            offs = np.zeros(3, np.int64)
            acc = 0
            for ax in axes:
                offs[ax] = acc
                acc += int(span[ax])
            tcol = np.arange(ps)
            for ax in axes:
                flat = offs[ax] + (pts_int[idx, ax] - lo[ax])
                OH[flat % S2, flat // S2, fcol + tcol] = FP8_NP(MPEN)
            for ax in axes:
                sa = int(span[ax])
                blo = np.maximum(means_int[gsel, ax] - radii[gsel] - lo[ax], 0)
                bhi = np.minimum(means_int[gsel, ax] + radii[gsel] - lo[ax], sa - 1)
                k = np.arange(sa)[:, None]
                box = ((k >= blo[None, :]) & (k <= bhi[None, :]))
                flat = offs[ax] + np.arange(sa)
                OH[flat % S2, flat // S2, gcol:gcol + gl] = np.where(
                    box, FP8_NP(1.0), FP8_NP(0.0))
            OPA[:gl, goff[s], :] = opac[gsel].astype(ml_dtypes.bfloat16)
            if gts[s] > 1:
                # split gsel across tiles (gl>128)
                OPA[:, goff[s]:goff[s] + gts[s], :] = 0
                for t in range(gts[s]):
                    seg = gsel[128 * t:128 * (t + 1)]
                    OPA[:len(seg), goff[s] + t, :] = opac[seg].astype(
                        ml_dtypes.bfloat16)
                # redo coefficient columns per tile
                # (handled above only for t=0; rebuild full block)
                for r in range(R):
                    FQ[r, gcol:gcol + gpad] = 0
                OH[:, :, gcol:gcol + gpad] = FP8_NP(0.0)
                for t in range(gts[s]):
                    seg = np.arange(128 * t, min(128 * (t + 1), gl))
                    gc2 = gcol + 128 * t
                    n2 = len(seg)
                    for r in range(base_rows):
                        wp = _tsplit(coefs[r][seg])
                        for i in range(6):
                            FQ[i * base_rows + r, gc2:gc2 + n2] = wp[WCOMBO[i]]
                    for i in range(6):
                        if WCOMBO[i] == 0:
                            FQ[i * base_rows + cr, gc2 + n2:gc2 + 128] = BF16(
                                -3.0 * MPEN)
                    for ax in axes:
                        sa = int(span[ax])
                        blo = np.maximum(
                            means_int[gsel[seg], ax] - radii[gsel[seg]] - lo[ax], 0)
                        bhi = np.minimum(
                            means_int[gsel[seg], ax] + radii[gsel[seg]] - lo[ax],
                            sa - 1)
                        k = np.arange(sa)[:, None]
                        box = ((k >= blo[None, :]) & (k <= bhi[None, :]))
                        flat = offs[ax] + np.arange(sa)
                        OH[flat % S2, flat // S2, gc2:gc2 + n2] = np.where(
                            box, FP8_NP(1.0), FP8_NP(0.0))

        ob = np.zeros((128, OBB), np.uint8)
        ob[:, :NG * n_c * 2] = OPA.reshape(128, NG * n_c).view(np.uint8)

        # pack the byte bundle: [fqW | ohW (per-tile) | per-shard fq+oh]
        SR = max(R, S2)
        sh_off = []
        acc = 2 * NG * 256
        for ps in PATTERN:
            sh_off.append(acc)
            acc += 4 * ps
        BUN = np.zeros((SR, acc), np.uint8)
        BUN[:R, 0:NG * 256] = np.ascontiguousarray(
            FQ[:, 0:128 * NG]).view(np.uint8)
        ohw = np.ascontiguousarray(
            OH[:, :, 0:128 * NG].reshape(S2, 2, NG, 128).transpose(0, 2, 1, 3))
        BUN[:S2, NG * 256:2 * NG * 256] = ohw.reshape(S2, NG * 256).view(
            np.uint8)
        for s in range(NS):
            ps = PATTERN[s]
            o = sh_off[s]
            BUN[:R, o:o + 2 * ps] = np.ascontiguousarray(
                FQ[:, CFW + poff[s]:CFW + poff[s] + ps]).view(np.uint8)
            BUN[:S2, o + 2 * ps:o + 4 * ps] = np.ascontiguousarray(
                OH[:, :, COW + poff[s]:COW + poff[s] + ps]).reshape(
                    S2, 2 * ps).view(np.uint8)
        in_maps.append({"bun": BUN, "ob": ob})

    perm = np.concatenate([cores[ci] for ci in range(N_CORES)])
    cfg = (R, S2, PATTERN, gts, n_c)
    return in_maps, perm, cfg


def _run(inputs, trace=False, **run_kwargs):
    in_maps, perm, cfg = _prepare(inputs)
    if cfg not in _nc_cache:
        _nc_cache[cfg] = _build_bass(*cfg)
    nc = _nc_cache[cfg]
    try:
        res = run_bass_kernel_spmd(
            nc, in_maps, core_ids=list(range(N_CORES)), trace=trace, **run_kwargs
        )
    except ModuleNotFoundError:
        res = run_bass_kernel_spmd(
            nc, in_maps, core_ids=list(range(N_CORES)), trace=False, **run_kwargs
        )
    P = P_CORE * N_CORES
    n_c = cfg[4]
    out = np.empty((P, n_c), np.float32)
    for ci in range(N_CORES):
        o = res.results[ci]["out"]  # [128, NB, C]
        out[perm[ci * P_CORE:(ci + 1) * P_CORE]] = (
            o.transpose(1, 0, 2).reshape(P_CORE, n_c))
    return out, res


def kernel(**inputs):
    return _run(inputs)[0]
